# revision 22
# baseline (speedup 1.0000x reference)
"""Trainium2 Bass kernel for dynamic per-sample depthwise conv (DPAC).

Reference computation (B=32, C=384, H=W=56, K=7):
  x_avg = mean(x, HW); x_max = max(x, HW)
  x_w   = gelu(x_avg @ w_avg.T + b_avg + x_max @ w_max.T + b_max)
  xr    = x_w[:,:,None] * w_mix  -> [B,C,49]
  Gx    = ||xr||_2 over taps = |x_w| * ||w_mix||_row
  Nx    = Gx / (mean_c(Gx) + eps)
  kern  = gamma * (xr * Nx) + beta  -> [B,C,7,7]
  out   = depthwise_conv(x, kern, pad=3)

Sharding: pure data parallel, batch split across 8 cores, params replicated.
Host-side work is layout only: zero-padding x for the conv halo and
pre-transposing/stacking the 1x1-conv weight matrices for the PE.
"""

import numpy as np
from contextlib import ExitStack

import concourse.bass as bass
import concourse.bass_isa as bass_isa
import concourse.tile as tile
from concourse import mybir
from concourse import bass_utils

B, C, H, W, KW = 32, 384, 56, 56, 7
NCORES = 8
BL = B // NCORES            # samples per core
PAD = KW // 2               # 3
HP, WP = H + 2 * PAD, W + 2 * PAD   # 62, 62
P = 128                     # partitions
CG = C // P                 # channel groups (3)
NTAPS = KW * KW             # 49
EPS = 1e-6

F32 = mybir.dt.float32
AX = mybir.AxisListType
OP = mybir.AluOpType
AF = mybir.ActivationFunctionType


def _prune_redundant_dma_waits(nc):
    """Drop DMA sem waits that are transitively implied by another wait.

    The dynamic-DMA descriptor carries only one sem wait (walrus rejects
    more: "Too many sync wait commands"), but Tile's wait assignment is
    per-proc minimal, not transitively minimal (optimize_sems is disabled),
    so recycled-slot DMAs get both a reader-engine wait and the old writer's
    DMA-lane wait even when the former implies the latter.

    Soundness: a wait (P >= t) on an in-order proc P (engine/sequencer)
    guarantees that P's instructions with tick <= t have completed, hence
    their own waits were satisfied; knowledge propagates transitively.
    DMA-lane procs are NOT assumed in-order: a lane wait only contributes
    its own fact.
    """
    import bass_rust as _br
    PROC_NAMES = _br.PROC_NAMES
    name_to_idx = {n: i for i, n in enumerate(PROC_NAMES)}
    dma_procs = {i for i, n in enumerate(PROC_NAMES) if n.startswith("DMA")}
    INC = {i: (16 if i in dma_procs else 1) for i in range(len(PROC_NAMES))}

    def sem_proc(ant_name):
        base = ant_name.rsplit("_", 1)[0]
        return name_to_idx.get(base)

    # per-proc streams in tick order
    streams = {}
    all_insts = []
    for blk in nc.m.functions[0].blocks:
        for ins in blk.instructions:
            all_insts.append(ins)
            p = ins.bass_scheduled_proc
            t = ins.bass_scheduled_tick
            if p is not None and t is not None:
                streams.setdefault(p, []).append((t, ins))
    for p in streams:
        streams[p].sort(key=lambda x: x[0])

    def merge(a, b):
        for k, v in b.items():
            if a.get(k, -1) < v:
                a[k] = v

    # K(ins): facts known when ins starts = from its waits.
    # SK(p, i): cumulative facts after first i insts of in-order proc p.
    K_memo, SK_memo = {}, {}

    def K(ins):
        r = K_memo.get(ins.name)
        if r is not None:
            return r
        K_memo[ins.name] = {}  # cycle guard (schedule is acyclic anyway)
        facts = {}
        si = ins.sync_info
        if si is not None:
            for w in si.on_wait:
                if w.wait_mode != "sem-ge-imm" or w.wait_reg is not None:
                    continue
                p = sem_proc(w.ant_name)
                if p is None:
                    continue
                t = w.wait_value // INC[p]
                merge(facts, {p: t})
                if p not in dma_procs:
                    merge(facts, SK(p, t))
        K_memo[ins.name] = facts
        return facts

    def SK(p, t):
        # facts after in-order proc p's stream reached tick t
        st = streams.get(p, [])
        i = 0
        while i < len(st) and st[i][0] <= t:
            i += 1
        key = (p, i)
        r = SK_memo.get(key)
        if r is not None:
            return r
        SK_memo[key] = {}  # cycle guard
        if i == 0:
            facts = {}
        else:
            tick_i, ins_i = st[i - 1]
            facts = dict(SK(p, tick_i - 1))
            merge(facts, K(ins_i))
            merge(facts, {p: tick_i})
        SK_memo[key] = facts
        return facts

    def DK(p, t):
        # facts guaranteed when in-order proc p DISPATCHES past tick t:
        # union of K(inst_i) for tick_i <= t (no completion facts; earlier
        # instructions' waits were satisfied at their dispatch, even if
        # they have not completed yet)
        st = streams.get(p, [])
        i = 0
        while i < len(st) and st[i][0] <= t:
            i += 1
        key = ("DK", p, i)
        r = SK_memo.get(key)
        if r is not None:
            return r
        SK_memo[key] = {}
        if i == 0:
            facts = {}
        else:
            tick_i, ins_i = st[i - 1]
            facts = dict(DK(p, tick_i - 1))
            merge(facts, K(ins_i))
        SK_memo[key] = facts
        return facts

    def prune_inst(ins, strict_one):
        si = ins.sync_info
        if si is None or len(si.on_wait) <= 1:
            return 0
        waits = list(si.on_wait)
        if any(w.wait_mode != "sem-ge-imm" or w.wait_reg is not None
               for w in waits):
            if strict_one:
                raise RuntimeError(f"{ins.name}: non-imm wait on {ins.name}")
            return 0
        # implicit knowledge from same-proc program order
        base = {}
        p0, t0 = ins.bass_scheduled_proc, ins.bass_scheduled_tick
        if p0 is not None and t0 is not None and p0 not in dma_procs:
            base = dict(DK(p0, t0 - 1))
        # iteratively drop any wait implied by base + the other waits
        kept = list(waits)
        changed = True
        while changed and len(kept) > 1:
            changed = False
            for w in list(kept):
                others = [o for o in kept if o is not w]
                facts = dict(base)
                for o in others:
                    p = sem_proc(o.ant_name)
                    if p is None:
                        continue
                    t = o.wait_value // INC[p]
                    merge(facts, {p: t})
                    if p not in dma_procs:
                        merge(facts, SK(p, t))
                pw = sem_proc(w.ant_name)
                tw = w.wait_value // INC[pw] if pw is not None else None
                if pw is not None and facts.get(pw, -1) >= tw:
                    kept = others
                    changed = True
                    break
        if strict_one and len(kept) > 1:
            raise RuntimeError(
                f"{ins.name} ({type(ins).__name__}): cannot reduce waits to "
                "one: " + str([(w.ant_name, w.wait_value) for w in kept]))
        if len(kept) < len(waits):
            si.on_wait = kept
            ins.sync_info = si
            return 1
        return 0

    n_pruned = 0
    for ins in all_insts:
        if not ins.is_executable():
            continue
        n_pruned += prune_inst(ins, strict_one=not ins.is_sequencer_only())
    return n_pruned


def _build_bass():
    nc = bass.Bass("TRN2", target_bir_lowering=False, debug=False,
                   num_devices=NCORES)

    xp = nc.dram_tensor("xp", [BL, C, HP, WP], F32, kind="ExternalInput").ap()
    w2t = nc.dram_tensor("w2t", [2, CG, P, C], F32, kind="ExternalInput").ap()
    b2 = nc.dram_tensor("b2", [2, C], F32, kind="ExternalInput").ap()
    wmx = nc.dram_tensor("wmx", [C, NTAPS], F32, kind="ExternalInput").ap()
    gam = nc.dram_tensor("gam", [C], F32, kind="ExternalInput").ap()
    bet = nc.dram_tensor("bet", [C], F32, kind="ExternalInput").ap()
    out = nc.dram_tensor("out", [BL, C, H, W], F32, kind="ExternalOutput").ap()

    with tile.TileContext(nc) as tc, ExitStack() as ctx:
        singles = ctx.enter_context(tc.tile_pool(name="singles", bufs=1))
        # x loads ride SWDGE exclusively; their recycle waits are implied by
        # reader waits and pruned by _prune_redundant_dma_waits
        xtp = ctx.enter_context(tc.tile_pool(name="xtp", bufs=6))
        accp = ctx.enter_context(tc.tile_pool(name="accp", bufs=2))
        small = ctx.enter_context(tc.tile_pool(name="small", bufs=4))
        psum = ctx.enter_context(tc.tile_pool(name="psum", bufs=2, space="PSUM"))

        # ---- load params (once) ----
        w2t_sb = singles.tile([P, 2, CG, C], F32)
        nc.sync.dma_start(out=w2t_sb, in_=w2t.rearrange("s g p c -> p s g c"))
        b2_sb = singles.tile([P, 2, CG], F32)
        nc.sync.dma_start(out=b2_sb, in_=b2.rearrange("s (g p) -> p s g", p=P))
        wmx_sb = singles.tile([P, CG, NTAPS], F32)
        nc.sync.dma_start(out=wmx_sb, in_=wmx.rearrange("(g p) t -> p g t", p=P))
        gam_sb = singles.tile([P, CG], F32)
        nc.sync.dma_start(out=gam_sb, in_=gam.rearrange("(g p) -> p g", p=P))
        bet_sb = singles.tile([P, CG], F32)
        nc.sync.dma_start(out=bet_sb, in_=bet.rearrange("(g p) -> p g", p=P))

        # DVE observers of param loads: thread each load's completion into
        # DVE's knowledge so downstream instruction waits collapse to one sem
        # (ISA instructions carry at most one sync wait)
        w2t_obs = singles.tile([P, 3], F32)
        nc.vector.tensor_copy(out=w2t_obs[:, 0:1], in_=w2t_sb[:, 0, 0, 0:1])
        nc.vector.tensor_copy(out=w2t_obs[:, 1:2], in_=gam_sb[:, 0:1])
        nc.vector.tensor_copy(out=w2t_obs[:, 2:3], in_=bet_sb[:, 0:1])

        # bias sum b_avg + b_max  [P, CG]
        bsum = singles.tile([P, CG], F32)
        nc.vector.tensor_add(out=bsum, in0=b2_sb[:, 0, :], in1=b2_sb[:, 1, :])

        # row norms of w_mix: wn[c] = sqrt(sum_t w_mix[c,t]^2)  [P, CG]
        wn = singles.tile([P, CG], F32)
        for g in range(CG):
            sq = small.tile([P, NTAPS], F32, tag="sq49")
            nc.vector.tensor_mul(out=sq, in0=wmx_sb[:, g, :], in1=wmx_sb[:, g, :])
            nc.vector.tensor_reduce(out=wn[:, g:g + 1], in_=sq, axis=AX.X, op=OP.add)
        nc.scalar.activation(out=wn, in_=wn, func=AF.Sqrt)

        # ---- phase 1: pooling over each [P, HP*WP] tile ----
        # X2[p, s, g, b]: s=0 mean, s=1 max
        x2 = singles.tile([P, 2, CG, BL], F32)
        for b in range(BL):
            for g in range(CG):
                xt = xtp.tile([P, HP, WP], F32, tag="xt")
                nc.gpsimd.dma_start(out=xt, in_=xp[b, g * P:(g + 1) * P, :, :])
                ssum = small.tile([P, 1], F32, tag="ssum")
                nc.vector.tensor_reduce(out=ssum, in_=xt, axis=AX.XY, op=OP.add)
                nc.vector.tensor_scalar_mul(x2[:, 0, g, b:b + 1], ssum, 1.0 / (H * W))
                nc.vector.tensor_reduce(
                    out=x2[:, 1, g, b:b + 1],
                    in_=xt[:, PAD:PAD + H, PAD:PAD + W], axis=AX.XY, op=OP.max)

        # ---- phase 2: stats -> kern ----
        xw = singles.tile([P, CG, BL], F32)
        gx = singles.tile([P, CG, BL], F32)
        for m in range(CG):  # output-channel group
            ps = psum.tile([P, BL], F32, tag="ps_mm")
            k = 0
            for s in range(2):
                for g in range(CG):
                    nc.tensor.matmul(
                        ps, w2t_sb[:, s, g, m * P:(m + 1) * P], x2[:, s, g, :],
                        start=(k == 0), stop=(k == 5))
                    k += 1
            # x_w = gelu(mm + b_avg + b_max)
            nc.scalar.activation(out=xw[:, m, :], in_=ps, func=AF.Gelu,
                                 bias=bsum[:, m:m + 1], scale=1.0)
            # Gx = |x_w| * wn  (wn > 0)
            nc.scalar.activation(out=gx[:, m, :], in_=xw[:, m, :], func=AF.Abs,
                                 scale=wn[:, m:m + 1])

        # mean over channels via PE: ones^T @ gx sums partitions
        ones_col = singles.tile([P, 1], F32)
        nc.vector.memset(ones_col, 1.0)
        ones_row = singles.tile([1, P], F32)
        nc.vector.memset(ones_row, 1.0)
        ps_sum = psum.tile([1, CG * BL], F32, tag="ps_sum")
        nc.tensor.matmul(ps_sum, ones_col, gx.rearrange("p g b -> p (g b)"),
                         start=True, stop=True)
        gsum_sb = small.tile([1, CG, BL], F32, tag="gsum")
        nc.vector.tensor_copy(out=gsum_sb.rearrange("p g b -> p (g b)"),
                              in_=ps_sum)
        rb1 = small.tile([1, BL], F32, tag="rb1")
        nc.vector.tensor_reduce(out=rb1, in_=gsum_sb.rearrange("p g b -> p b g"),
                                axis=AX.X, op=OP.add)
        # r = 1 / (mean + eps)
        nc.vector.tensor_scalar(out=rb1, in0=rb1, scalar1=1.0 / C,
                                scalar2=EPS, op0=OP.mult, op1=OP.add)
        nc.vector.reciprocal(out=rb1, in_=rb1)
        # broadcast r to all partitions: ones_row^T @ rb1 -> [P, BL] in PSUM
        rb = psum.tile([P, BL], F32, tag="ps_rb")
        nc.tensor.matmul(rb, ones_row, rb1, start=True, stop=True)

        # s = gamma * x_w * Gx * r ; kern = w_mix * s + beta
        kern = singles.tile([P, CG, BL, NTAPS], F32)
        for m in range(CG):
            t = small.tile([P, BL], F32, tag="s_tmp")
            nc.vector.tensor_mul(out=t, in0=xw[:, m, :], in1=gx[:, m, :])
            nc.vector.tensor_mul(out=t, in0=t, in1=rb)
            nc.vector.tensor_scalar_mul(t, t, gam_sb[:, m:m + 1])
            for b in range(BL):
                nc.vector.tensor_scalar(
                    out=kern[:, m, b, :], in0=wmx_sb[:, m, :],
                    scalar1=t[:, b:b + 1], scalar2=bet_sb[:, m:m + 1],
                    op0=OP.mult, op1=OP.add)

        # ---- phase 3: depthwise conv, 49 shifted MAC taps ----
        # one store per sample (4 total) so HWDGE ring slots never recycle
        # into an unobservable store->store wait
        for b in range(BL):
            acc = accp.tile([P, CG, H, W], F32, tag="acc")
            for g in range(CG):
                xt = xtp.tile([P, HP, WP], F32, tag="xt")
                nc.gpsimd.dma_start(out=xt, in_=xp[b, g * P:(g + 1) * P, :, :])
                for t in range(NTAPS):
                    di, dj = t // KW, t % KW
                    xv = xt[:, di:di + H, dj:dj + W]
                    ks = kern[:, g, b, t:t + 1]
                    if t == 0:
                        nc.vector.tensor_scalar(out=acc[:, g], in0=xv,
                                                scalar1=ks, scalar2=None,
                                                op0=OP.mult)
                    else:
                        nc.vector.scalar_tensor_tensor(
                            out=acc[:, g], in0=xv, scalar=ks, in1=acc[:, g],
                            op0=OP.mult, op1=OP.add)
            nc.sync.dma_start(
                out=out[b].rearrange("(g p) h w -> p g h w", p=P), in_=acc)
            # DVE observer of the store: threads the store's completion into
            # DVE knowledge so the next slot writer needs no DMA-lane wait
            nc.vector.memset(acc[:, 0, 0, 0:1], 0.0)

    import sys
    sys.setrecursionlimit(100000)
    _prune_redundant_dma_waits(nc)
    return nc


_NC_CACHE = {}


def _get_nc():
    if "nc" not in _NC_CACHE:
        _NC_CACHE["nc"] = _build_bass()
    return _NC_CACHE["nc"]


def _prep_inputs(x, w_avg, b_avg, w_max, b_max, w_mix, gamma, beta):
    x = np.asarray(x, dtype=np.float32)
    xp = np.zeros((B, C, HP, WP), dtype=np.float32)
    xp[:, :, PAD:PAD + H, PAD:PAD + W] = x
    w2t = np.ascontiguousarray(
        np.stack([np.asarray(w_avg, np.float32).T.reshape(CG, P, C),
                  np.asarray(w_max, np.float32).T.reshape(CG, P, C)]))
    b2 = np.ascontiguousarray(
        np.stack([np.asarray(b_avg, np.float32), np.asarray(b_max, np.float32)]))
    shared = {
        "w2t": w2t,
        "b2": b2,
        "wmx": np.ascontiguousarray(np.asarray(w_mix, np.float32)),
        "gam": np.ascontiguousarray(np.asarray(gamma, np.float32).reshape(C)),
        "bet": np.ascontiguousarray(np.asarray(beta, np.float32).reshape(C)),
    }
    in_maps = []
    for i in range(NCORES):
        m = dict(shared)
        m["xp"] = np.ascontiguousarray(xp[i * BL:(i + 1) * BL])
        in_maps.append(m)
    return in_maps


def run(inputs, trace=False):
    nc = _get_nc()
    in_maps = _prep_inputs(**inputs)
    res = bass_utils.run_bass_kernel_spmd(
        nc, in_maps, core_ids=list(range(NCORES)), trace=trace)
    outs = [res.results[i]["out"] for i in range(NCORES)]
    full = np.concatenate(outs, axis=0).astype(np.float32)
    return full, res


def kernel(**inputs) -> np.ndarray:
    full, _ = run(inputs, trace=False)
    return full


# revision 34
# speedup vs baseline: 1.0245x; 1.0245x over previous
"""Trainium2 Bass kernel for dynamic per-sample depthwise conv (DPAC).

Reference computation (B=32, C=384, H=W=56, K=7):
  x_avg = mean(x, HW); x_max = max(x, HW)
  x_w   = gelu(x_avg @ w_avg.T + b_avg + x_max @ w_max.T + b_max)
  xr    = x_w[:,:,None] * w_mix  -> [B,C,49]
  Gx    = ||xr||_2 over taps = |x_w| * ||w_mix||_row
  Nx    = Gx / (mean_c(Gx) + eps)
  kern  = gamma * (xr * Nx) + beta  -> [B,C,7,7]
  out   = depthwise_conv(x, kern, pad=3)

Sharding: pure data parallel, batch split across 8 cores, params replicated.
Host-side work is layout only: zero-padding x for the conv halo and
pre-transposing/stacking the 1x1-conv weight matrices for the PE.
"""

import numpy as np
from contextlib import ExitStack

import concourse.bass as bass
import concourse.bass_isa as bass_isa
import concourse.tile as tile
from concourse import mybir
from concourse import bass_utils

B, C, H, W, KW = 32, 384, 56, 56, 7
NCORES = 8
BL = B // NCORES            # samples per core
PAD = KW // 2               # 3
HP, WP = H + 2 * PAD, W + 2 * PAD   # 62, 62
P = 128                     # partitions
CG = C // P                 # channel groups (3)
NTAPS = KW * KW             # 49
EPS = 1e-6

F32 = mybir.dt.float32
AX = mybir.AxisListType
OP = mybir.AluOpType
AF = mybir.ActivationFunctionType


def _prune_redundant_dma_waits(nc):
    """Drop DMA sem waits that are transitively implied by another wait.

    The dynamic-DMA descriptor carries only one sem wait (walrus rejects
    more: "Too many sync wait commands"), but Tile's wait assignment is
    per-proc minimal, not transitively minimal (optimize_sems is disabled),
    so recycled-slot DMAs get both a reader-engine wait and the old writer's
    DMA-lane wait even when the former implies the latter.

    Soundness: a wait (P >= t) on an in-order proc P (engine/sequencer)
    guarantees that P's instructions with tick <= t have completed, hence
    their own waits were satisfied; knowledge propagates transitively.
    DMA-lane procs are NOT assumed in-order: a lane wait only contributes
    its own fact.
    """
    import bass_rust as _br
    PROC_NAMES = _br.PROC_NAMES
    name_to_idx = {n: i for i, n in enumerate(PROC_NAMES)}
    dma_procs = {i for i, n in enumerate(PROC_NAMES) if n.startswith("DMA")}
    INC = {i: (16 if i in dma_procs else 1) for i in range(len(PROC_NAMES))}

    def sem_proc(ant_name):
        base = ant_name.rsplit("_", 1)[0]
        return name_to_idx.get(base)

    # per-proc streams in tick order
    streams = {}
    all_insts = []
    for blk in nc.m.functions[0].blocks:
        for ins in blk.instructions:
            all_insts.append(ins)
            p = ins.bass_scheduled_proc
            t = ins.bass_scheduled_tick
            if p is not None and t is not None:
                streams.setdefault(p, []).append((t, ins))
    for p in streams:
        streams[p].sort(key=lambda x: x[0])

    def merge(a, b):
        for k, v in b.items():
            if a.get(k, -1) < v:
                a[k] = v

    # K(ins): facts known when ins starts = from its waits.
    # SK(p, i): cumulative facts after first i insts of in-order proc p.
    K_memo, SK_memo = {}, {}

    def K(ins):
        r = K_memo.get(ins.name)
        if r is not None:
            return r
        K_memo[ins.name] = {}  # cycle guard (schedule is acyclic anyway)
        facts = {}
        si = ins.sync_info
        if si is not None:
            for w in si.on_wait:
                if w.wait_mode != "sem-ge-imm" or w.wait_reg is not None:
                    continue
                p = sem_proc(w.ant_name)
                if p is None:
                    continue
                t = w.wait_value // INC[p]
                merge(facts, {p: t})
                if p not in dma_procs:
                    merge(facts, SK(p, t))
                else:
                    # lane sem >= 16t implies the first t DMAs on the lane
                    # started (per-lane FIFO), hence their waits held
                    merge(facts, DK(p, t))
        K_memo[ins.name] = facts
        return facts

    def SK(p, t):
        # facts after in-order proc p's stream reached tick t
        st = streams.get(p, [])
        i = 0
        while i < len(st) and st[i][0] <= t:
            i += 1
        key = (p, i)
        r = SK_memo.get(key)
        if r is not None:
            return r
        SK_memo[key] = {}  # cycle guard
        if i == 0:
            facts = {}
        else:
            tick_i, ins_i = st[i - 1]
            facts = dict(SK(p, tick_i - 1))
            merge(facts, K(ins_i))
            merge(facts, {p: tick_i})
        SK_memo[key] = facts
        return facts

    def DK(p, t):
        # facts guaranteed when in-order proc p DISPATCHES past tick t:
        # union of K(inst_i) for tick_i <= t (no completion facts; earlier
        # instructions' waits were satisfied at their dispatch, even if
        # they have not completed yet)
        st = streams.get(p, [])
        i = 0
        while i < len(st) and st[i][0] <= t:
            i += 1
        key = ("DK", p, i)
        r = SK_memo.get(key)
        if r is not None:
            return r
        SK_memo[key] = {}
        if i == 0:
            facts = {}
        else:
            tick_i, ins_i = st[i - 1]
            facts = dict(DK(p, tick_i - 1))
            merge(facts, K(ins_i))
        SK_memo[key] = facts
        return facts

    def prune_inst(ins, strict_one):
        si = ins.sync_info
        if si is None or len(si.on_wait) <= 1:
            return 0
        waits = list(si.on_wait)
        if any(w.wait_mode != "sem-ge-imm" or w.wait_reg is not None
               for w in waits):
            if strict_one:
                raise RuntimeError(f"{ins.name}: non-imm wait on {ins.name}")
            return 0
        # implicit knowledge from same-proc program order
        base = {}
        p0, t0 = ins.bass_scheduled_proc, ins.bass_scheduled_tick
        if p0 is not None and t0 is not None and p0 not in dma_procs:
            base = dict(DK(p0, t0 - 1))
            # DVE/ACT/PE execute and complete strictly in order with an
            # output-hazard interlock, so same-engine RAW data is visible
            # without an explicit sem wait
            # Pool (gpsimd) qualifies for partition-aligned elementwise
            # chains: each Q7 core runs its instruction stream in order and
            # only touches its own 16 partitions
            if PROC_NAMES[p0] in ("DVE", "Activation", "PE", "Pool"):
                merge(base, SK(p0, t0 - 1))
                merge(base, {p0: t0 - 1})
        # iteratively drop any wait implied by base + the other waits
        kept = list(waits)
        changed = True
        while changed and len(kept) > 1:
            changed = False
            for w in list(kept):
                others = [o for o in kept if o is not w]
                facts = dict(base)
                for o in others:
                    p = sem_proc(o.ant_name)
                    if p is None:
                        continue
                    t = o.wait_value // INC[p]
                    merge(facts, {p: t})
                    if p not in dma_procs:
                        merge(facts, SK(p, t))
                    else:
                        merge(facts, DK(p, t))
                pw = sem_proc(w.ant_name)
                tw = w.wait_value // INC[pw] if pw is not None else None
                if pw is not None and facts.get(pw, -1) >= tw:
                    kept = others
                    changed = True
                    break
        if strict_one and len(kept) > 1:
            raise RuntimeError(
                f"{ins.name} ({type(ins).__name__}): cannot reduce waits to "
                "one: " + str([(w.ant_name, w.wait_value) for w in kept]))
        if len(kept) < len(waits):
            si.on_wait = kept
            ins.sync_info = si
            return 1
        return 0

    n_pruned = 0
    for ins in all_insts:
        if not ins.is_executable():
            continue
        n_pruned += prune_inst(ins, strict_one=not ins.is_sequencer_only())
    return n_pruned


def _build_bass():
    nc = bass.Bass("TRN2", target_bir_lowering=False, debug=False,
                   num_devices=NCORES)

    xp = nc.dram_tensor("xp", [BL, C, HP, WP], F32, kind="ExternalInput").ap()
    w2t = nc.dram_tensor("w2t", [2, CG, P, C], F32, kind="ExternalInput").ap()
    b2 = nc.dram_tensor("b2", [2, C], F32, kind="ExternalInput").ap()
    wmx = nc.dram_tensor("wmx", [C, NTAPS], F32, kind="ExternalInput").ap()
    gam = nc.dram_tensor("gam", [C], F32, kind="ExternalInput").ap()
    bet = nc.dram_tensor("bet", [C], F32, kind="ExternalInput").ap()
    out = nc.dram_tensor("out", [BL, C, H, W], F32, kind="ExternalOutput").ap()

    with tile.TileContext(nc) as tc, ExitStack() as ctx:
        singles = ctx.enter_context(tc.tile_pool(name="singles", bufs=1))
        xtp = ctx.enter_context(tc.tile_pool(name="xtp", bufs=4))
        accp = ctx.enter_context(tc.tile_pool(name="accp", bufs=2))
        small = ctx.enter_context(tc.tile_pool(name="small", bufs=4))
        psum = ctx.enter_context(tc.tile_pool(name="psum", bufs=2, space="PSUM"))

        # ---- load params (once) ----
        w2t_sb = singles.tile([P, 2, CG, C], F32)
        nc.sync.dma_start(out=w2t_sb, in_=w2t.rearrange("s g p c -> p s g c"))
        b2_sb = singles.tile([P, 2, CG], F32)
        nc.sync.dma_start(out=b2_sb, in_=b2.rearrange("s (g p) -> p s g", p=P))
        wmx_sb = singles.tile([P, CG, NTAPS], F32)
        nc.sync.dma_start(out=wmx_sb, in_=wmx.rearrange("(g p) t -> p g t", p=P))
        gam_sb = singles.tile([P, CG], F32)
        nc.sync.dma_start(out=gam_sb, in_=gam.rearrange("(g p) -> p g", p=P))
        bet_sb = singles.tile([P, CG], F32)
        nc.sync.dma_start(out=bet_sb, in_=bet.rearrange("(g p) -> p g", p=P))

        # DVE observers of param loads: thread each load's completion into
        # DVE's knowledge so downstream instruction waits collapse to one sem
        # (ISA instructions carry at most one sync wait)
        w2t_obs = singles.tile([P, 3], F32)
        nc.vector.tensor_copy(out=w2t_obs[:, 0:1], in_=w2t_sb[:, 0, 0, 0:1])
        nc.vector.tensor_copy(out=w2t_obs[:, 1:2], in_=gam_sb[:, 0:1])
        nc.vector.tensor_copy(out=w2t_obs[:, 2:3], in_=bet_sb[:, 0:1])

        # bias sum b_avg + b_max  [P, CG]
        bsum = singles.tile([P, CG], F32)
        nc.vector.tensor_add(out=bsum, in0=b2_sb[:, 0, :], in1=b2_sb[:, 1, :])

        # row norms of w_mix: wn[c] = sqrt(sum_t w_mix[c,t]^2)  [P, CG]
        wn = singles.tile([P, CG], F32)
        for g in range(CG):
            sq = small.tile([P, NTAPS], F32, tag="sq49")
            nc.vector.tensor_mul(out=sq, in0=wmx_sb[:, g, :], in1=wmx_sb[:, g, :])
            nc.vector.tensor_reduce(out=wn[:, g:g + 1], in_=sq, axis=AX.X, op=OP.add)
        nc.scalar.activation(out=wn, in_=wn, func=AF.Sqrt)

        # ---- phase 1: pooling over each [P, HP*WP] tile ----
        # X2[p, s, g, b]: s=0 mean, s=1 max
        x2 = singles.tile([P, 2, CG, BL], F32)
        for b in range(BL):
            for g in range(CG):
                xt = xtp.tile([P, HP, WP], F32, tag="xt")
                nc.sync.dma_start(out=xt, in_=xp[b, g * P:(g + 1) * P, :, :])
                ssum = small.tile([P, 1], F32, tag="ssum")
                nc.vector.tensor_reduce(out=ssum, in_=xt, axis=AX.XY, op=OP.add)
                nc.vector.tensor_scalar_mul(x2[:, 0, g, b:b + 1], ssum, 1.0 / (H * W))
                nc.vector.tensor_reduce(
                    out=x2[:, 1, g, b:b + 1],
                    in_=xt[:, PAD:PAD + H, PAD:PAD + W], axis=AX.XY, op=OP.max)

        # ---- phase 2: stats -> kern ----
        xw = singles.tile([P, CG, BL], F32)
        gx = singles.tile([P, CG, BL], F32)
        for m in range(CG):  # output-channel group
            ps = psum.tile([P, BL], F32, tag="ps_mm")
            k = 0
            for s in range(2):
                for g in range(CG):
                    nc.tensor.matmul(
                        ps, w2t_sb[:, s, g, m * P:(m + 1) * P], x2[:, s, g, :],
                        start=(k == 0), stop=(k == 5))
                    k += 1
            # x_w = gelu(mm + b_avg + b_max)
            nc.scalar.activation(out=xw[:, m, :], in_=ps, func=AF.Gelu,
                                 bias=bsum[:, m:m + 1], scale=1.0)
            # Gx = |x_w| * wn  (wn > 0)
            nc.scalar.activation(out=gx[:, m, :], in_=xw[:, m, :], func=AF.Abs,
                                 scale=wn[:, m:m + 1])

        # mean over channels via PE: ones^T @ gx sums partitions
        ones_col = singles.tile([P, 1], F32)
        nc.vector.memset(ones_col, 1.0)
        ones_row = singles.tile([1, P], F32)
        nc.vector.memset(ones_row, 1.0)
        ps_sum = psum.tile([1, CG * BL], F32, tag="ps_sum")
        nc.tensor.matmul(ps_sum, ones_col, gx.rearrange("p g b -> p (g b)"),
                         start=True, stop=True)
        gsum_sb = small.tile([1, CG, BL], F32, tag="gsum")
        nc.vector.tensor_copy(out=gsum_sb.rearrange("p g b -> p (g b)"),
                              in_=ps_sum)
        rb1 = small.tile([1, BL], F32, tag="rb1")
        nc.vector.tensor_reduce(out=rb1, in_=gsum_sb.rearrange("p g b -> p b g"),
                                axis=AX.X, op=OP.add)
        # r = 1 / (mean + eps)
        nc.vector.tensor_scalar(out=rb1, in0=rb1, scalar1=1.0 / C,
                                scalar2=EPS, op0=OP.mult, op1=OP.add)
        nc.vector.reciprocal(out=rb1, in_=rb1)
        # broadcast r to all partitions: ones_row^T @ rb1 -> [P, BL] in PSUM
        rb = psum.tile([P, BL], F32, tag="ps_rb")
        nc.tensor.matmul(rb, ones_row, rb1, start=True, stop=True)

        # s = gamma * x_w * Gx * r ; kern = w_mix * s + beta
        kern = singles.tile([P, CG, BL, NTAPS], F32)
        for m in range(CG):
            t = small.tile([P, BL], F32, tag="s_tmp")
            nc.vector.tensor_mul(out=t, in0=xw[:, m, :], in1=gx[:, m, :])
            nc.vector.tensor_mul(out=t, in0=t, in1=rb)
            nc.vector.tensor_scalar_mul(t, t, gam_sb[:, m:m + 1])
            for b in range(BL):
                nc.vector.tensor_scalar(
                    out=kern[:, m, b, :], in0=wmx_sb[:, m, :],
                    scalar1=t[:, b:b + 1], scalar2=bet_sb[:, m:m + 1],
                    op0=OP.mult, op1=OP.add)

        # ---- phase 3: depthwise conv, 49 shifted MAC taps ----
        # 3-way engine split: DVE runs fused STT taps; ACT pre-scales taps
        # (per-partition scale) into tmp tiles; GPSIMD accumulates those with
        # plain tensor_tensor adds into its own accumulator.
        ACT_TAPS = list(range(1, 2 * 16, 2))   # 16 taps via ACT+GPS
        for b in range(BL):
            for g in range(CG):
                xt = xtp.tile([P, HP, WP], F32, tag="xt")
                nc.sync.dma_start(out=xt, in_=xp[b, g * P:(g + 1) * P, :, :])
                # per-engine observers of the load: collapse downstream waits
                obs_v = small.tile([P, 1], F32, tag="obs_v")
                nc.vector.tensor_copy(out=obs_v, in_=xt[:, 0, 0:1])
                obs_a = small.tile([P, 1], F32, tag="obs_a")
                nc.scalar.copy(out=obs_a, in_=xt[:, 0, 0:1])
                # ACT observer of this tile's kern column (DVE-written)
                obs_k = small.tile([P, 1], F32, tag="obs_k")
                nc.scalar.copy(out=obs_k, in_=kern[:, g, b, 0:1])
                acc = accp.tile([P, H, W], F32, tag="acc")
                accg = accp.tile([P, H, W], F32, tag="accg")
                first_v, first_g = True, True
                tmp = None
                for t in range(NTAPS):
                    di, dj = t // KW, t % KW
                    xv = xt[:, di:di + H, dj:dj + W]
                    ks = kern[:, g, b, t:t + 1]
                    if t in ACT_TAPS:
                        tmp = accp.tile([P, H, W], F32, tag="tmp_act")
                        nc.scalar.mul(out=tmp, in_=xv, mul=ks)
                        if first_g:
                            nc.gpsimd.tensor_copy(out=accg, in_=tmp)
                            first_g = False
                        else:
                            nc.gpsimd.tensor_add(out=accg, in0=accg, in1=tmp)
                    elif first_v:
                        nc.vector.tensor_scalar(out=acc, in0=xv, scalar1=ks,
                                                scalar2=None, op0=OP.mult)
                        first_v = False
                    else:
                        nc.vector.scalar_tensor_tensor(
                            out=acc, in0=xv, scalar=ks, in1=acc,
                            op0=OP.mult, op1=OP.add)
                # combine on gpsimd: accg stays engine-local
                nc.gpsimd.tensor_add(out=acc, in0=acc, in1=accg)
                # ACT tail observer, then a DVE join reading xt + the ACT
                # tail: the slot-recycling load then needs only one wait
                obs_a2 = small.tile([P, 1], F32, tag="obs_a2")
                nc.scalar.copy(out=obs_a2, in_=tmp[:, 0, 0:1])
                nc.vector.tensor_add(out=obs_v, in0=xt[:, 0, 0:1],
                                     in1=obs_a2)
                nc.sync.dma_start(out=out[b, g * P:(g + 1) * P, :, :],
                                  in_=acc)
                # DVE observer of the store completion for slot recycling
                nc.vector.memset(acc[:, 0, 0:1], 0.0)

    import sys
    sys.setrecursionlimit(100000)
    _prune_redundant_dma_waits(nc)
    return nc


_NC_CACHE = {}


def _get_nc():
    if "nc" not in _NC_CACHE:
        _NC_CACHE["nc"] = _build_bass()
    return _NC_CACHE["nc"]


def _prep_inputs(x, w_avg, b_avg, w_max, b_max, w_mix, gamma, beta):
    x = np.asarray(x, dtype=np.float32)
    xp = np.zeros((B, C, HP, WP), dtype=np.float32)
    xp[:, :, PAD:PAD + H, PAD:PAD + W] = x
    w2t = np.ascontiguousarray(
        np.stack([np.asarray(w_avg, np.float32).T.reshape(CG, P, C),
                  np.asarray(w_max, np.float32).T.reshape(CG, P, C)]))
    b2 = np.ascontiguousarray(
        np.stack([np.asarray(b_avg, np.float32), np.asarray(b_max, np.float32)]))
    shared = {
        "w2t": w2t,
        "b2": b2,
        "wmx": np.ascontiguousarray(np.asarray(w_mix, np.float32)),
        "gam": np.ascontiguousarray(np.asarray(gamma, np.float32).reshape(C)),
        "bet": np.ascontiguousarray(np.asarray(beta, np.float32).reshape(C)),
    }
    in_maps = []
    for i in range(NCORES):
        m = dict(shared)
        m["xp"] = np.ascontiguousarray(xp[i * BL:(i + 1) * BL])
        in_maps.append(m)
    return in_maps


def run(inputs, trace=False):
    nc = _get_nc()
    in_maps = _prep_inputs(**inputs)
    res = bass_utils.run_bass_kernel_spmd(
        nc, in_maps, core_ids=list(range(NCORES)), trace=trace)
    outs = [res.results[i]["out"] for i in range(NCORES)]
    full = np.concatenate(outs, axis=0).astype(np.float32)
    return full, res


def kernel(**inputs) -> np.ndarray:
    full, _ = run(inputs, trace=False)
    return full


# revision 35
# speedup vs baseline: 1.0273x; 1.0028x over previous
"""Trainium2 Bass kernel for dynamic per-sample depthwise conv (DPAC).

Reference computation (B=32, C=384, H=W=56, K=7):
  x_avg = mean(x, HW); x_max = max(x, HW)
  x_w   = gelu(x_avg @ w_avg.T + b_avg + x_max @ w_max.T + b_max)
  xr    = x_w[:,:,None] * w_mix  -> [B,C,49]
  Gx    = ||xr||_2 over taps = |x_w| * ||w_mix||_row
  Nx    = Gx / (mean_c(Gx) + eps)
  kern  = gamma * (xr * Nx) + beta  -> [B,C,7,7]
  out   = depthwise_conv(x, kern, pad=3)

Sharding: pure data parallel, batch split across 8 cores, params replicated.
Host-side work is layout only: zero-padding x for the conv halo and
pre-transposing/stacking the 1x1-conv weight matrices for the PE.
"""

import numpy as np
from contextlib import ExitStack

import concourse.bass as bass
import concourse.bass_isa as bass_isa
import concourse.tile as tile
from concourse import mybir
from concourse import bass_utils

B, C, H, W, KW = 32, 384, 56, 56, 7
NCORES = 8
BL = B // NCORES            # samples per core
PAD = KW // 2               # 3
HP, WP = H + 2 * PAD, W + 2 * PAD   # 62, 62
P = 128                     # partitions
CG = C // P                 # channel groups (3)
NTAPS = KW * KW             # 49
EPS = 1e-6

F32 = mybir.dt.float32
AX = mybir.AxisListType
OP = mybir.AluOpType
AF = mybir.ActivationFunctionType


def _prune_redundant_dma_waits(nc):
    """Drop DMA sem waits that are transitively implied by another wait.

    The dynamic-DMA descriptor carries only one sem wait (walrus rejects
    more: "Too many sync wait commands"), but Tile's wait assignment is
    per-proc minimal, not transitively minimal (optimize_sems is disabled),
    so recycled-slot DMAs get both a reader-engine wait and the old writer's
    DMA-lane wait even when the former implies the latter.

    Soundness: a wait (P >= t) on an in-order proc P (engine/sequencer)
    guarantees that P's instructions with tick <= t have completed, hence
    their own waits were satisfied; knowledge propagates transitively.
    DMA-lane procs are NOT assumed in-order: a lane wait only contributes
    its own fact.
    """
    import bass_rust as _br
    PROC_NAMES = _br.PROC_NAMES
    name_to_idx = {n: i for i, n in enumerate(PROC_NAMES)}
    dma_procs = {i for i, n in enumerate(PROC_NAMES) if n.startswith("DMA")}
    INC = {i: (16 if i in dma_procs else 1) for i in range(len(PROC_NAMES))}

    def sem_proc(ant_name):
        base = ant_name.rsplit("_", 1)[0]
        return name_to_idx.get(base)

    # per-proc streams in tick order
    streams = {}
    all_insts = []
    for blk in nc.m.functions[0].blocks:
        for ins in blk.instructions:
            all_insts.append(ins)
            p = ins.bass_scheduled_proc
            t = ins.bass_scheduled_tick
            if p is not None and t is not None:
                streams.setdefault(p, []).append((t, ins))
    for p in streams:
        streams[p].sort(key=lambda x: x[0])

    def merge(a, b):
        for k, v in b.items():
            if a.get(k, -1) < v:
                a[k] = v

    # K(ins): facts known when ins starts = from its waits.
    # SK(p, i): cumulative facts after first i insts of in-order proc p.
    K_memo, SK_memo = {}, {}

    def K(ins):
        r = K_memo.get(ins.name)
        if r is not None:
            return r
        K_memo[ins.name] = {}  # cycle guard (schedule is acyclic anyway)
        facts = {}
        si = ins.sync_info
        if si is not None:
            for w in si.on_wait:
                if w.wait_mode != "sem-ge-imm" or w.wait_reg is not None:
                    continue
                p = sem_proc(w.ant_name)
                if p is None:
                    continue
                t = w.wait_value // INC[p]
                merge(facts, {p: t})
                if p not in dma_procs:
                    merge(facts, SK(p, t))
                else:
                    # lane sem >= 16t implies the first t DMAs on the lane
                    # started (per-lane FIFO), hence their waits held
                    merge(facts, DK(p, t))
        K_memo[ins.name] = facts
        return facts

    def SK(p, t):
        # facts after in-order proc p's stream reached tick t
        st = streams.get(p, [])
        i = 0
        while i < len(st) and st[i][0] <= t:
            i += 1
        key = (p, i)
        r = SK_memo.get(key)
        if r is not None:
            return r
        SK_memo[key] = {}  # cycle guard
        if i == 0:
            facts = {}
        else:
            tick_i, ins_i = st[i - 1]
            facts = dict(SK(p, tick_i - 1))
            merge(facts, K(ins_i))
            merge(facts, {p: tick_i})
        SK_memo[key] = facts
        return facts

    def DK(p, t):
        # facts guaranteed when in-order proc p DISPATCHES past tick t:
        # union of K(inst_i) for tick_i <= t (no completion facts; earlier
        # instructions' waits were satisfied at their dispatch, even if
        # they have not completed yet)
        st = streams.get(p, [])
        i = 0
        while i < len(st) and st[i][0] <= t:
            i += 1
        key = ("DK", p, i)
        r = SK_memo.get(key)
        if r is not None:
            return r
        SK_memo[key] = {}
        if i == 0:
            facts = {}
        else:
            tick_i, ins_i = st[i - 1]
            facts = dict(DK(p, tick_i - 1))
            merge(facts, K(ins_i))
        SK_memo[key] = facts
        return facts

    def prune_inst(ins, strict_one):
        si = ins.sync_info
        if si is None or len(si.on_wait) <= 1:
            return 0
        waits = list(si.on_wait)
        if any(w.wait_mode != "sem-ge-imm" or w.wait_reg is not None
               for w in waits):
            if strict_one:
                raise RuntimeError(f"{ins.name}: non-imm wait on {ins.name}")
            return 0
        # implicit knowledge from same-proc program order
        base = {}
        p0, t0 = ins.bass_scheduled_proc, ins.bass_scheduled_tick
        if p0 is not None and t0 is not None and p0 not in dma_procs:
            base = dict(DK(p0, t0 - 1))
            # DVE/ACT/PE execute and complete strictly in order with an
            # output-hazard interlock, so same-engine RAW data is visible
            # without an explicit sem wait
            # Pool (gpsimd) qualifies for partition-aligned elementwise
            # chains: each Q7 core runs its instruction stream in order and
            # only touches its own 16 partitions
            if PROC_NAMES[p0] in ("DVE", "Activation", "PE", "Pool"):
                merge(base, SK(p0, t0 - 1))
                merge(base, {p0: t0 - 1})
        # iteratively drop any wait implied by base + the other waits
        kept = list(waits)
        changed = True
        while changed and len(kept) > 1:
            changed = False
            for w in list(kept):
                others = [o for o in kept if o is not w]
                facts = dict(base)
                for o in others:
                    p = sem_proc(o.ant_name)
                    if p is None:
                        continue
                    t = o.wait_value // INC[p]
                    merge(facts, {p: t})
                    if p not in dma_procs:
                        merge(facts, SK(p, t))
                    else:
                        merge(facts, DK(p, t))
                pw = sem_proc(w.ant_name)
                tw = w.wait_value // INC[pw] if pw is not None else None
                if pw is not None and facts.get(pw, -1) >= tw:
                    kept = others
                    changed = True
                    break
        if strict_one and len(kept) > 1:
            raise RuntimeError(
                f"{ins.name} ({type(ins).__name__}): cannot reduce waits to "
                "one: " + str([(w.ant_name, w.wait_value) for w in kept]))
        if len(kept) < len(waits):
            si.on_wait = kept
            ins.sync_info = si
            return 1
        return 0

    n_pruned = 0
    for ins in all_insts:
        if not ins.is_executable():
            continue
        n_pruned += prune_inst(ins, strict_one=not ins.is_sequencer_only())
    return n_pruned


def _build_bass():
    nc = bass.Bass("TRN2", target_bir_lowering=False, debug=False,
                   num_devices=NCORES)

    xp = nc.dram_tensor("xp", [BL, C, HP, WP], F32, kind="ExternalInput").ap()
    w2t = nc.dram_tensor("w2t", [2, CG, P, C], F32, kind="ExternalInput").ap()
    b2 = nc.dram_tensor("b2", [2, C], F32, kind="ExternalInput").ap()
    wmx = nc.dram_tensor("wmx", [C, NTAPS], F32, kind="ExternalInput").ap()
    gam = nc.dram_tensor("gam", [C], F32, kind="ExternalInput").ap()
    bet = nc.dram_tensor("bet", [C], F32, kind="ExternalInput").ap()
    out = nc.dram_tensor("out", [BL, C, H, W], F32, kind="ExternalOutput").ap()

    with tile.TileContext(nc) as tc, ExitStack() as ctx:
        singles = ctx.enter_context(tc.tile_pool(name="singles", bufs=1))
        xtp = ctx.enter_context(tc.tile_pool(name="xtp", bufs=4))
        accp = ctx.enter_context(tc.tile_pool(name="accp", bufs=2))
        tmpp = ctx.enter_context(tc.tile_pool(name="tmpp", bufs=4))
        small = ctx.enter_context(tc.tile_pool(name="small", bufs=4))
        psum = ctx.enter_context(tc.tile_pool(name="psum", bufs=2, space="PSUM"))

        # ---- load params (once) ----
        w2t_sb = singles.tile([P, 2, CG, C], F32)
        nc.sync.dma_start(out=w2t_sb, in_=w2t.rearrange("s g p c -> p s g c"))
        b2_sb = singles.tile([P, 2, CG], F32)
        nc.sync.dma_start(out=b2_sb, in_=b2.rearrange("s (g p) -> p s g", p=P))
        wmx_sb = singles.tile([P, CG, NTAPS], F32)
        nc.sync.dma_start(out=wmx_sb, in_=wmx.rearrange("(g p) t -> p g t", p=P))
        gam_sb = singles.tile([P, CG], F32)
        nc.sync.dma_start(out=gam_sb, in_=gam.rearrange("(g p) -> p g", p=P))
        bet_sb = singles.tile([P, CG], F32)
        nc.sync.dma_start(out=bet_sb, in_=bet.rearrange("(g p) -> p g", p=P))

        # DVE observers of param loads: thread each load's completion into
        # DVE's knowledge so downstream instruction waits collapse to one sem
        # (ISA instructions carry at most one sync wait)
        w2t_obs = singles.tile([P, 3], F32)
        nc.vector.tensor_copy(out=w2t_obs[:, 0:1], in_=w2t_sb[:, 0, 0, 0:1])
        nc.vector.tensor_copy(out=w2t_obs[:, 1:2], in_=gam_sb[:, 0:1])
        nc.vector.tensor_copy(out=w2t_obs[:, 2:3], in_=bet_sb[:, 0:1])

        # bias sum b_avg + b_max  [P, CG]
        bsum = singles.tile([P, CG], F32)
        nc.vector.tensor_add(out=bsum, in0=b2_sb[:, 0, :], in1=b2_sb[:, 1, :])

        # row norms of w_mix: wn[c] = sqrt(sum_t w_mix[c,t]^2)  [P, CG]
        wn = singles.tile([P, CG], F32)
        for g in range(CG):
            sq = small.tile([P, NTAPS], F32, tag="sq49")
            nc.vector.tensor_mul(out=sq, in0=wmx_sb[:, g, :], in1=wmx_sb[:, g, :])
            nc.vector.tensor_reduce(out=wn[:, g:g + 1], in_=sq, axis=AX.X, op=OP.add)
        nc.scalar.activation(out=wn, in_=wn, func=AF.Sqrt)

        # ---- phase 1: pooling over each [P, HP*WP] tile ----
        # X2[p, s, g, b]: s=0 mean, s=1 max
        x2 = singles.tile([P, 2, CG, BL], F32)
        for b in range(BL):
            for g in range(CG):
                xt = xtp.tile([P, HP, WP], F32, tag="xt")
                nc.sync.dma_start(out=xt, in_=xp[b, g * P:(g + 1) * P, :, :])
                ssum = small.tile([P, 1], F32, tag="ssum")
                nc.vector.tensor_reduce(out=ssum, in_=xt, axis=AX.XY, op=OP.add)
                nc.vector.tensor_scalar_mul(x2[:, 0, g, b:b + 1], ssum, 1.0 / (H * W))
                nc.vector.tensor_reduce(
                    out=x2[:, 1, g, b:b + 1],
                    in_=xt[:, PAD:PAD + H, PAD:PAD + W], axis=AX.XY, op=OP.max)

        # ---- phase 2: stats -> kern ----
        xw = singles.tile([P, CG, BL], F32)
        gx = singles.tile([P, CG, BL], F32)
        for m in range(CG):  # output-channel group
            ps = psum.tile([P, BL], F32, tag="ps_mm")
            k = 0
            for s in range(2):
                for g in range(CG):
                    nc.tensor.matmul(
                        ps, w2t_sb[:, s, g, m * P:(m + 1) * P], x2[:, s, g, :],
                        start=(k == 0), stop=(k == 5))
                    k += 1
            # x_w = gelu(mm + b_avg + b_max)
            nc.scalar.activation(out=xw[:, m, :], in_=ps, func=AF.Gelu,
                                 bias=bsum[:, m:m + 1], scale=1.0)
            # Gx = |x_w| * wn  (wn > 0)
            nc.scalar.activation(out=gx[:, m, :], in_=xw[:, m, :], func=AF.Abs,
                                 scale=wn[:, m:m + 1])

        # mean over channels via PE: ones^T @ gx sums partitions
        ones_col = singles.tile([P, 1], F32)
        nc.vector.memset(ones_col, 1.0)
        ones_row = singles.tile([1, P], F32)
        nc.vector.memset(ones_row, 1.0)
        ps_sum = psum.tile([1, CG * BL], F32, tag="ps_sum")
        nc.tensor.matmul(ps_sum, ones_col, gx.rearrange("p g b -> p (g b)"),
                         start=True, stop=True)
        gsum_sb = small.tile([1, CG, BL], F32, tag="gsum")
        nc.vector.tensor_copy(out=gsum_sb.rearrange("p g b -> p (g b)"),
                              in_=ps_sum)
        rb1 = small.tile([1, BL], F32, tag="rb1")
        nc.vector.tensor_reduce(out=rb1, in_=gsum_sb.rearrange("p g b -> p b g"),
                                axis=AX.X, op=OP.add)
        # r = 1 / (mean + eps)
        nc.vector.tensor_scalar(out=rb1, in0=rb1, scalar1=1.0 / C,
                                scalar2=EPS, op0=OP.mult, op1=OP.add)
        nc.vector.reciprocal(out=rb1, in_=rb1)
        # broadcast r to all partitions: ones_row^T @ rb1 -> [P, BL] in PSUM
        rb = psum.tile([P, BL], F32, tag="ps_rb")
        nc.tensor.matmul(rb, ones_row, rb1, start=True, stop=True)

        # s = gamma * x_w * Gx * r ; kern = w_mix * s + beta
        kern = singles.tile([P, CG, BL, NTAPS], F32)
        for m in range(CG):
            t = small.tile([P, BL], F32, tag="s_tmp")
            nc.vector.tensor_mul(out=t, in0=xw[:, m, :], in1=gx[:, m, :])
            nc.vector.tensor_mul(out=t, in0=t, in1=rb)
            nc.vector.tensor_scalar_mul(t, t, gam_sb[:, m:m + 1])
            for b in range(BL):
                nc.vector.tensor_scalar(
                    out=kern[:, m, b, :], in0=wmx_sb[:, m, :],
                    scalar1=t[:, b:b + 1], scalar2=bet_sb[:, m:m + 1],
                    op0=OP.mult, op1=OP.add)

        # ---- phase 3: depthwise conv, 49 shifted MAC taps ----
        # 3-way engine split: DVE runs fused STT taps; ACT pre-scales taps
        # (per-partition scale) into tmp tiles; GPSIMD accumulates those with
        # plain tensor_tensor adds into its own accumulator.
        ACT_TAPS = list(range(1, 2 * 16, 2))   # 16 taps via ACT+GPS
        for b in range(BL):
            for g in range(CG):
                xt = xtp.tile([P, HP, WP], F32, tag="xt")
                nc.sync.dma_start(out=xt, in_=xp[b, g * P:(g + 1) * P, :, :])
                # per-engine observers of the load: collapse downstream waits
                obs_v = small.tile([P, 1], F32, tag="obs_v")
                nc.vector.tensor_copy(out=obs_v, in_=xt[:, 0, 0:1])
                obs_a = small.tile([P, 1], F32, tag="obs_a")
                nc.scalar.copy(out=obs_a, in_=xt[:, 0, 0:1])
                # ACT observer of this tile's kern column (DVE-written)
                obs_k = small.tile([P, 1], F32, tag="obs_k")
                nc.scalar.copy(out=obs_k, in_=kern[:, g, b, 0:1])
                acc = accp.tile([P, H, W], F32, tag="acc")
                accg = accp.tile([P, H, W], F32, tag="accg")
                first_v, first_g = True, True
                tmp = None
                for t in range(NTAPS):
                    di, dj = t // KW, t % KW
                    xv = xt[:, di:di + H, dj:dj + W]
                    ks = kern[:, g, b, t:t + 1]
                    if t in ACT_TAPS:
                        tmp = tmpp.tile([P, H, W], F32, tag="tmp_act")
                        nc.scalar.mul(out=tmp, in_=xv, mul=ks)
                        if first_g:
                            nc.gpsimd.tensor_copy(out=accg, in_=tmp)
                            first_g = False
                        else:
                            nc.gpsimd.tensor_add(out=accg, in0=accg, in1=tmp)
                    elif first_v:
                        nc.vector.tensor_scalar(out=acc, in0=xv, scalar1=ks,
                                                scalar2=None, op0=OP.mult)
                        first_v = False
                    else:
                        nc.vector.scalar_tensor_tensor(
                            out=acc, in0=xv, scalar=ks, in1=acc,
                            op0=OP.mult, op1=OP.add)
                # combine on gpsimd: accg stays engine-local
                nc.gpsimd.tensor_add(out=acc, in0=acc, in1=accg)
                # ACT tail observer, then a DVE join reading xt + the ACT
                # tail: the slot-recycling load then needs only one wait
                obs_a2 = small.tile([P, 1], F32, tag="obs_a2")
                nc.scalar.copy(out=obs_a2, in_=tmp[:, 0, 0:1])
                nc.vector.tensor_add(out=obs_v, in0=xt[:, 0, 0:1],
                                     in1=obs_a2)
                nc.sync.dma_start(out=out[b, g * P:(g + 1) * P, :, :],
                                  in_=acc)
                # DVE observer of the store completion for slot recycling
                nc.vector.memset(acc[:, 0, 0:1], 0.0)

    import sys
    sys.setrecursionlimit(100000)
    _prune_redundant_dma_waits(nc)
    return nc


_NC_CACHE = {}


def _get_nc():
    if "nc" not in _NC_CACHE:
        _NC_CACHE["nc"] = _build_bass()
    return _NC_CACHE["nc"]


def _prep_inputs(x, w_avg, b_avg, w_max, b_max, w_mix, gamma, beta):
    x = np.asarray(x, dtype=np.float32)
    xp = np.zeros((B, C, HP, WP), dtype=np.float32)
    xp[:, :, PAD:PAD + H, PAD:PAD + W] = x
    w2t = np.ascontiguousarray(
        np.stack([np.asarray(w_avg, np.float32).T.reshape(CG, P, C),
                  np.asarray(w_max, np.float32).T.reshape(CG, P, C)]))
    b2 = np.ascontiguousarray(
        np.stack([np.asarray(b_avg, np.float32), np.asarray(b_max, np.float32)]))
    shared = {
        "w2t": w2t,
        "b2": b2,
        "wmx": np.ascontiguousarray(np.asarray(w_mix, np.float32)),
        "gam": np.ascontiguousarray(np.asarray(gamma, np.float32).reshape(C)),
        "bet": np.ascontiguousarray(np.asarray(beta, np.float32).reshape(C)),
    }
    in_maps = []
    for i in range(NCORES):
        m = dict(shared)
        m["xp"] = np.ascontiguousarray(xp[i * BL:(i + 1) * BL])
        in_maps.append(m)
    return in_maps


def run(inputs, trace=False):
    nc = _get_nc()
    in_maps = _prep_inputs(**inputs)
    res = bass_utils.run_bass_kernel_spmd(
        nc, in_maps, core_ids=list(range(NCORES)), trace=trace)
    outs = [res.results[i]["out"] for i in range(NCORES)]
    full = np.concatenate(outs, axis=0).astype(np.float32)
    return full, res


def kernel(**inputs) -> np.ndarray:
    full, _ = run(inputs, trace=False)
    return full


# revision 36
# speedup vs baseline: 6231.2008x; 6065.5047x over previous
"""Trainium2 Bass kernel for dynamic per-sample depthwise conv (DPAC).

Reference computation (B=32, C=384, H=W=56, K=7):
  x_avg = mean(x, HW); x_max = max(x, HW)
  x_w   = gelu(x_avg @ w_avg.T + b_avg + x_max @ w_max.T + b_max)
  xr    = x_w[:,:,None] * w_mix  -> [B,C,49]
  Gx    = ||xr||_2 over taps = |x_w| * ||w_mix||_row
  Nx    = Gx / (mean_c(Gx) + eps)
  kern  = gamma * (xr * Nx) + beta  -> [B,C,7,7]
  out   = depthwise_conv(x, kern, pad=3)

Sharding: pure data parallel, batch split across 8 cores, params replicated.
Host-side work is layout only: zero-padding x for the conv halo and
pre-transposing/stacking the 1x1-conv weight matrices for the PE.
"""

import numpy as np
from contextlib import ExitStack

import concourse.bass as bass
import concourse.bass_isa as bass_isa
import concourse.tile as tile
from concourse import mybir
from concourse import bass_utils

B, C, H, W, KW = 32, 384, 56, 56, 7
NCORES = 8
BL = B // NCORES            # samples per core
PAD = KW // 2               # 3
HP, WP = H + 2 * PAD, W + 2 * PAD   # 62, 62
P = 128                     # partitions
CG = C // P                 # channel groups (3)
NTAPS = KW * KW             # 49
EPS = 1e-6

F32 = mybir.dt.float32
AX = mybir.AxisListType
OP = mybir.AluOpType
AF = mybir.ActivationFunctionType


def _prune_redundant_dma_waits(nc):
    """Drop DMA sem waits that are transitively implied by another wait.

    The dynamic-DMA descriptor carries only one sem wait (walrus rejects
    more: "Too many sync wait commands"), but Tile's wait assignment is
    per-proc minimal, not transitively minimal (optimize_sems is disabled),
    so recycled-slot DMAs get both a reader-engine wait and the old writer's
    DMA-lane wait even when the former implies the latter.

    Soundness: a wait (P >= t) on an in-order proc P (engine/sequencer)
    guarantees that P's instructions with tick <= t have completed, hence
    their own waits were satisfied; knowledge propagates transitively.
    DMA-lane procs are NOT assumed in-order: a lane wait only contributes
    its own fact.
    """
    import bass_rust as _br
    PROC_NAMES = _br.PROC_NAMES
    name_to_idx = {n: i for i, n in enumerate(PROC_NAMES)}
    dma_procs = {i for i, n in enumerate(PROC_NAMES) if n.startswith("DMA")}
    INC = {i: (16 if i in dma_procs else 1) for i in range(len(PROC_NAMES))}

    def sem_proc(ant_name):
        base = ant_name.rsplit("_", 1)[0]
        return name_to_idx.get(base)

    # per-proc streams in tick order
    streams = {}
    all_insts = []
    for blk in nc.m.functions[0].blocks:
        for ins in blk.instructions:
            all_insts.append(ins)
            p = ins.bass_scheduled_proc
            t = ins.bass_scheduled_tick
            if p is not None and t is not None:
                streams.setdefault(p, []).append((t, ins))
    for p in streams:
        streams[p].sort(key=lambda x: x[0])

    def merge(a, b):
        for k, v in b.items():
            if a.get(k, -1) < v:
                a[k] = v

    # K(ins): facts known when ins starts = from its waits.
    # SK(p, i): cumulative facts after first i insts of in-order proc p.
    K_memo, SK_memo = {}, {}

    def K(ins):
        r = K_memo.get(ins.name)
        if r is not None:
            return r
        K_memo[ins.name] = {}  # cycle guard (schedule is acyclic anyway)
        facts = {}
        si = ins.sync_info
        if si is not None:
            for w in si.on_wait:
                if w.wait_mode != "sem-ge-imm" or w.wait_reg is not None:
                    continue
                p = sem_proc(w.ant_name)
                if p is None:
                    continue
                t = w.wait_value // INC[p]
                merge(facts, {p: t})
                if p not in dma_procs:
                    merge(facts, SK(p, t))
                else:
                    # lane sem >= 16t implies the first t DMAs on the lane
                    # started (per-lane FIFO), hence their waits held
                    merge(facts, DK(p, t))
        K_memo[ins.name] = facts
        return facts

    def SK(p, t):
        # facts after in-order proc p's stream reached tick t
        st = streams.get(p, [])
        i = 0
        while i < len(st) and st[i][0] <= t:
            i += 1
        key = (p, i)
        r = SK_memo.get(key)
        if r is not None:
            return r
        SK_memo[key] = {}  # cycle guard
        if i == 0:
            facts = {}
        else:
            tick_i, ins_i = st[i - 1]
            facts = dict(SK(p, tick_i - 1))
            merge(facts, K(ins_i))
            merge(facts, {p: tick_i})
        SK_memo[key] = facts
        return facts

    def DK(p, t):
        # facts guaranteed when in-order proc p DISPATCHES past tick t:
        # union of K(inst_i) for tick_i <= t (no completion facts; earlier
        # instructions' waits were satisfied at their dispatch, even if
        # they have not completed yet)
        st = streams.get(p, [])
        i = 0
        while i < len(st) and st[i][0] <= t:
            i += 1
        key = ("DK", p, i)
        r = SK_memo.get(key)
        if r is not None:
            return r
        SK_memo[key] = {}
        if i == 0:
            facts = {}
        else:
            tick_i, ins_i = st[i - 1]
            facts = dict(DK(p, tick_i - 1))
            merge(facts, K(ins_i))
        SK_memo[key] = facts
        return facts

    def prune_inst(ins, strict_one):
        si = ins.sync_info
        if si is None or len(si.on_wait) <= 1:
            return 0
        waits = list(si.on_wait)
        if any(w.wait_mode != "sem-ge-imm" or w.wait_reg is not None
               for w in waits):
            if strict_one:
                raise RuntimeError(f"{ins.name}: non-imm wait on {ins.name}")
            return 0
        # implicit knowledge from same-proc program order
        base = {}
        p0, t0 = ins.bass_scheduled_proc, ins.bass_scheduled_tick
        if p0 is not None and t0 is not None and p0 not in dma_procs:
            base = dict(DK(p0, t0 - 1))
            # DVE/ACT/PE execute and complete strictly in order with an
            # output-hazard interlock, so same-engine RAW data is visible
            # without an explicit sem wait
            # Pool (gpsimd) qualifies for partition-aligned elementwise
            # chains: each Q7 core runs its instruction stream in order and
            # only touches its own 16 partitions
            if PROC_NAMES[p0] in ("DVE", "Activation", "PE", "Pool"):
                merge(base, SK(p0, t0 - 1))
                merge(base, {p0: t0 - 1})
        # iteratively drop any wait implied by base + the other waits
        kept = list(waits)
        changed = True
        while changed and len(kept) > 1:
            changed = False
            for w in list(kept):
                others = [o for o in kept if o is not w]
                facts = dict(base)
                for o in others:
                    p = sem_proc(o.ant_name)
                    if p is None:
                        continue
                    t = o.wait_value // INC[p]
                    merge(facts, {p: t})
                    if p not in dma_procs:
                        merge(facts, SK(p, t))
                    else:
                        merge(facts, DK(p, t))
                pw = sem_proc(w.ant_name)
                tw = w.wait_value // INC[pw] if pw is not None else None
                if pw is not None and facts.get(pw, -1) >= tw:
                    kept = others
                    changed = True
                    break
        if strict_one and len(kept) > 1:
            raise RuntimeError(
                f"{ins.name} ({type(ins).__name__}): cannot reduce waits to "
                "one: " + str([(w.ant_name, w.wait_value) for w in kept]))
        if len(kept) < len(waits):
            si.on_wait = kept
            ins.sync_info = si
            return 1
        return 0

    n_pruned = 0
    for ins in all_insts:
        if not ins.is_executable():
            continue
        n_pruned += prune_inst(ins, strict_one=not ins.is_sequencer_only())
    return n_pruned


def _build_bass():
    nc = bass.Bass("TRN2", target_bir_lowering=False, debug=False,
                   num_devices=NCORES)

    xp = nc.dram_tensor("xp", [BL, C, HP, WP], F32, kind="ExternalInput").ap()
    w2t = nc.dram_tensor("w2t", [2, CG, P, C], F32, kind="ExternalInput").ap()
    b2 = nc.dram_tensor("b2", [2, C], F32, kind="ExternalInput").ap()
    wmx = nc.dram_tensor("wmx", [C, NTAPS], F32, kind="ExternalInput").ap()
    gam = nc.dram_tensor("gam", [C], F32, kind="ExternalInput").ap()
    bet = nc.dram_tensor("bet", [C], F32, kind="ExternalInput").ap()
    out = nc.dram_tensor("out", [BL, C, H, W], F32, kind="ExternalOutput").ap()

    with tile.TileContext(nc) as tc, ExitStack() as ctx:
        singles = ctx.enter_context(tc.tile_pool(name="singles", bufs=1))
        xtp = ctx.enter_context(tc.tile_pool(name="xtp", bufs=4))
        accp = ctx.enter_context(tc.tile_pool(name="accp", bufs=3))
        tmpp = ctx.enter_context(tc.tile_pool(name="tmpp", bufs=3))
        small = ctx.enter_context(tc.tile_pool(name="small", bufs=4))
        psum = ctx.enter_context(tc.tile_pool(name="psum", bufs=2, space="PSUM"))

        # ---- load params (once) ----
        w2t_sb = singles.tile([P, 2, CG, C], F32)
        nc.sync.dma_start(out=w2t_sb, in_=w2t.rearrange("s g p c -> p s g c"))
        b2_sb = singles.tile([P, 2, CG], F32)
        nc.sync.dma_start(out=b2_sb, in_=b2.rearrange("s (g p) -> p s g", p=P))
        wmx_sb = singles.tile([P, CG, NTAPS], F32)
        nc.sync.dma_start(out=wmx_sb, in_=wmx.rearrange("(g p) t -> p g t", p=P))
        gam_sb = singles.tile([P, CG], F32)
        nc.sync.dma_start(out=gam_sb, in_=gam.rearrange("(g p) -> p g", p=P))
        bet_sb = singles.tile([P, CG], F32)
        nc.sync.dma_start(out=bet_sb, in_=bet.rearrange("(g p) -> p g", p=P))

        # DVE observers of param loads: thread each load's completion into
        # DVE's knowledge so downstream instruction waits collapse to one sem
        # (ISA instructions carry at most one sync wait)
        w2t_obs = singles.tile([P, 3], F32)
        nc.vector.tensor_copy(out=w2t_obs[:, 0:1], in_=w2t_sb[:, 0, 0, 0:1])
        nc.vector.tensor_copy(out=w2t_obs[:, 1:2], in_=gam_sb[:, 0:1])
        nc.vector.tensor_copy(out=w2t_obs[:, 2:3], in_=bet_sb[:, 0:1])

        # bias sum b_avg + b_max  [P, CG]
        bsum = singles.tile([P, CG], F32)
        nc.vector.tensor_add(out=bsum, in0=b2_sb[:, 0, :], in1=b2_sb[:, 1, :])

        # row norms of w_mix: wn[c] = sqrt(sum_t w_mix[c,t]^2)  [P, CG]
        wn = singles.tile([P, CG], F32)
        for g in range(CG):
            sq = small.tile([P, NTAPS], F32, tag="sq49")
            nc.vector.tensor_mul(out=sq, in0=wmx_sb[:, g, :], in1=wmx_sb[:, g, :])
            nc.vector.tensor_reduce(out=wn[:, g:g + 1], in_=sq, axis=AX.X, op=OP.add)
        nc.scalar.activation(out=wn, in_=wn, func=AF.Sqrt)

        # ---- phase 1: pooling over each [P, HP*WP] tile ----
        # X2[p, s, g, b]: s=0 mean, s=1 max
        x2 = singles.tile([P, 2, CG, BL], F32)
        for b in range(BL):
            for g in range(CG):
                xt = xtp.tile([P, HP, WP], F32, tag="xt")
                nc.sync.dma_start(out=xt, in_=xp[b, g * P:(g + 1) * P, :, :])
                ssum = small.tile([P, 1], F32, tag="ssum")
                nc.vector.tensor_reduce(out=ssum, in_=xt, axis=AX.XY, op=OP.add)
                nc.vector.tensor_scalar_mul(x2[:, 0, g, b:b + 1], ssum, 1.0 / (H * W))
                nc.vector.tensor_reduce(
                    out=x2[:, 1, g, b:b + 1],
                    in_=xt[:, PAD:PAD + H, PAD:PAD + W], axis=AX.XY, op=OP.max)

        # ---- phase 2: stats -> kern ----
        xw = singles.tile([P, CG, BL], F32)
        gx = singles.tile([P, CG, BL], F32)
        for m in range(CG):  # output-channel group
            ps = psum.tile([P, BL], F32, tag="ps_mm")
            k = 0
            for s in range(2):
                for g in range(CG):
                    nc.tensor.matmul(
                        ps, w2t_sb[:, s, g, m * P:(m + 1) * P], x2[:, s, g, :],
                        start=(k == 0), stop=(k == 5))
                    k += 1
            # x_w = gelu(mm + b_avg + b_max)
            nc.scalar.activation(out=xw[:, m, :], in_=ps, func=AF.Gelu,
                                 bias=bsum[:, m:m + 1], scale=1.0)
            # Gx = |x_w| * wn  (wn > 0)
            nc.scalar.activation(out=gx[:, m, :], in_=xw[:, m, :], func=AF.Abs,
                                 scale=wn[:, m:m + 1])

        # mean over channels via PE: ones^T @ gx sums partitions
        ones_col = singles.tile([P, 1], F32)
        nc.vector.memset(ones_col, 1.0)
        ones_row = singles.tile([1, P], F32)
        nc.vector.memset(ones_row, 1.0)
        ps_sum = psum.tile([1, CG * BL], F32, tag="ps_sum")
        nc.tensor.matmul(ps_sum, ones_col, gx.rearrange("p g b -> p (g b)"),
                         start=True, stop=True)
        gsum_sb = small.tile([1, CG, BL], F32, tag="gsum")
        nc.vector.tensor_copy(out=gsum_sb.rearrange("p g b -> p (g b)"),
                              in_=ps_sum)
        rb1 = small.tile([1, BL], F32, tag="rb1")
        nc.vector.tensor_reduce(out=rb1, in_=gsum_sb.rearrange("p g b -> p b g"),
                                axis=AX.X, op=OP.add)
        # r = 1 / (mean + eps)
        nc.vector.tensor_scalar(out=rb1, in0=rb1, scalar1=1.0 / C,
                                scalar2=EPS, op0=OP.mult, op1=OP.add)
        nc.vector.reciprocal(out=rb1, in_=rb1)
        # broadcast r to all partitions: ones_row^T @ rb1 -> [P, BL] in PSUM
        rb = psum.tile([P, BL], F32, tag="ps_rb")
        nc.tensor.matmul(rb, ones_row, rb1, start=True, stop=True)

        # s = gamma * x_w * Gx * r ; kern = w_mix * s + beta
        kern = singles.tile([P, CG, BL, NTAPS], F32)
        for m in range(CG):
            t = small.tile([P, BL], F32, tag="s_tmp")
            nc.vector.tensor_mul(out=t, in0=xw[:, m, :], in1=gx[:, m, :])
            nc.vector.tensor_mul(out=t, in0=t, in1=rb)
            nc.vector.tensor_scalar_mul(t, t, gam_sb[:, m:m + 1])
            for b in range(BL):
                nc.vector.tensor_scalar(
                    out=kern[:, m, b, :], in0=wmx_sb[:, m, :],
                    scalar1=t[:, b:b + 1], scalar2=bet_sb[:, m:m + 1],
                    op0=OP.mult, op1=OP.add)

        # ---- phase 3: depthwise conv, 49 shifted MAC taps ----
        # 3-way engine split: DVE runs fused STT taps; ACT pre-scales taps
        # (per-partition scale) into tmp tiles; GPSIMD accumulates those with
        # plain tensor_tensor adds into its own accumulator.
        ACT_TAPS = list(range(1, 2 * 16, 2))   # 16 taps via ACT+GPS
        for b in range(BL):
            for g in range(CG):
                xt = xtp.tile([P, HP, WP], F32, tag="xt")
                nc.sync.dma_start(out=xt, in_=xp[b, g * P:(g + 1) * P, :, :])
                # per-engine observers of the load: collapse downstream waits
                obs_v = small.tile([P, 1], F32, tag="obs_v")
                nc.vector.tensor_copy(out=obs_v, in_=xt[:, 0, 0:1])
                obs_a = small.tile([P, 1], F32, tag="obs_a")
                nc.scalar.copy(out=obs_a, in_=xt[:, 0, 0:1])
                # ACT observer of this tile's kern column (DVE-written)
                obs_k = small.tile([P, 1], F32, tag="obs_k")
                nc.scalar.copy(out=obs_k, in_=kern[:, g, b, 0:1])
                acc = accp.tile([P, H, W], F32, tag="acc")
                accg = accp.tile([P, H, W], F32, tag="accg")
                first_v, first_g = True, True
                tmp = None
                for t in range(NTAPS):
                    di, dj = t // KW, t % KW
                    xv = xt[:, di:di + H, dj:dj + W]
                    ks = kern[:, g, b, t:t + 1]
                    if t in ACT_TAPS:
                        tmp = tmpp.tile([P, H, W], F32, tag="tmp_act")
                        nc.scalar.mul(out=tmp, in_=xv, mul=ks)
                        if first_g:
                            nc.gpsimd.tensor_copy(out=accg, in_=tmp)
                            first_g = False
                        else:
                            nc.gpsimd.tensor_add(out=accg, in0=accg, in1=tmp)
                    elif first_v:
                        nc.vector.tensor_scalar(out=acc, in0=xv, scalar1=ks,
                                                scalar2=None, op0=OP.mult)
                        first_v = False
                    else:
                        nc.vector.scalar_tensor_tensor(
                            out=acc, in0=xv, scalar=ks, in1=acc,
                            op0=OP.mult, op1=OP.add)
                # combine on gpsimd: accg stays engine-local
                nc.gpsimd.tensor_add(out=acc, in0=acc, in1=accg)
                # ACT tail observer, then a DVE join reading xt + the ACT
                # tail: the slot-recycling load then needs only one wait
                obs_a2 = small.tile([P, 1], F32, tag="obs_a2")
                nc.scalar.copy(out=obs_a2, in_=tmp[:, 0, 0:1])
                nc.vector.tensor_add(out=obs_v, in0=xt[:, 0, 0:1],
                                     in1=obs_a2)
                nc.sync.dma_start(out=out[b, g * P:(g + 1) * P, :, :],
                                  in_=acc)
                # DVE observer of the store completion for slot recycling
                nc.vector.memset(acc[:, 0, 0:1], 0.0)

    import sys
    sys.setrecursionlimit(100000)
    _prune_redundant_dma_waits(nc)
    return nc


_NC_CACHE = {}


def _get_nc():
    if "nc" not in _NC_CACHE:
        _NC_CACHE["nc"] = _build_bass()
    return _NC_CACHE["nc"]


def _prep_inputs(x, w_avg, b_avg, w_max, b_max, w_mix, gamma, beta):
    x = np.asarray(x, dtype=np.float32)
    xp = np.zeros((B, C, HP, WP), dtype=np.float32)
    xp[:, :, PAD:PAD + H, PAD:PAD + W] = x
    w2t = np.ascontiguousarray(
        np.stack([np.asarray(w_avg, np.float32).T.reshape(CG, P, C),
                  np.asarray(w_max, np.float32).T.reshape(CG, P, C)]))
    b2 = np.ascontiguousarray(
        np.stack([np.asarray(b_avg, np.float32), np.asarray(b_max, np.float32)]))
    shared = {
        "w2t": w2t,
        "b2": b2,
        "wmx": np.ascontiguousarray(np.asarray(w_mix, np.float32)),
        "gam": np.ascontiguousarray(np.asarray(gamma, np.float32).reshape(C)),
        "bet": np.ascontiguousarray(np.asarray(beta, np.float32).reshape(C)),
    }
    in_maps = []
    for i in range(NCORES):
        m = dict(shared)
        m["xp"] = np.ascontiguousarray(xp[i * BL:(i + 1) * BL])
        in_maps.append(m)
    return in_maps


def run(inputs, trace=False):
    nc = _get_nc()
    in_maps = _prep_inputs(**inputs)
    res = bass_utils.run_bass_kernel_spmd(
        nc, in_maps, core_ids=list(range(NCORES)), trace=trace)
    outs = [res.results[i]["out"] for i in range(NCORES)]
    full = np.concatenate(outs, axis=0).astype(np.float32)
    return full, res


def kernel(**inputs) -> np.ndarray:
    full, _ = run(inputs, trace=False)
    return full


# revision 41
# speedup vs baseline: 6314.5859x; 1.0134x over previous
"""Trainium2 Bass kernel for dynamic per-sample depthwise conv (DPAC).

Reference computation (B=32, C=384, H=W=56, K=7):
  x_avg = mean(x, HW); x_max = max(x, HW)
  x_w   = gelu(x_avg @ w_avg.T + b_avg + x_max @ w_max.T + b_max)
  xr    = x_w[:,:,None] * w_mix  -> [B,C,49]
  Gx    = ||xr||_2 over taps = |x_w| * ||w_mix||_row
  Nx    = Gx / (mean_c(Gx) + eps)
  kern  = gamma * (xr * Nx) + beta  -> [B,C,7,7]
  out   = depthwise_conv(x, kern, pad=3)

Sharding: pure data parallel, batch split across 8 cores, params replicated.
Host-side work is layout only: zero-padding x for the conv halo and
pre-transposing/stacking the 1x1-conv weight matrices for the PE.
"""

import numpy as np
from contextlib import ExitStack

import concourse.bass as bass
import concourse.bass_isa as bass_isa
import concourse.tile as tile
from concourse import mybir
from concourse import bass_utils

B, C, H, W, KW = 32, 384, 56, 56, 7
NCORES = 8
BL = B // NCORES            # samples per core
PAD = KW // 2               # 3
HP, WP = H + 2 * PAD, W + 2 * PAD   # 62, 62
P = 128                     # partitions
CG = C // P                 # channel groups (3)
NTAPS = KW * KW             # 49
EPS = 1e-6

F32 = mybir.dt.float32
AX = mybir.AxisListType
OP = mybir.AluOpType
AF = mybir.ActivationFunctionType


def _prune_redundant_dma_waits(nc):
    """Drop DMA sem waits that are transitively implied by another wait.

    The dynamic-DMA descriptor carries only one sem wait (walrus rejects
    more: "Too many sync wait commands"), but Tile's wait assignment is
    per-proc minimal, not transitively minimal (optimize_sems is disabled),
    so recycled-slot DMAs get both a reader-engine wait and the old writer's
    DMA-lane wait even when the former implies the latter.

    Soundness: a wait (P >= t) on an in-order proc P (engine/sequencer)
    guarantees that P's instructions with tick <= t have completed, hence
    their own waits were satisfied; knowledge propagates transitively.
    DMA-lane procs are NOT assumed in-order: a lane wait only contributes
    its own fact.
    """
    import bass_rust as _br
    PROC_NAMES = _br.PROC_NAMES
    name_to_idx = {n: i for i, n in enumerate(PROC_NAMES)}
    dma_procs = {i for i, n in enumerate(PROC_NAMES) if n.startswith("DMA")}
    INC = {i: (16 if i in dma_procs else 1) for i in range(len(PROC_NAMES))}

    def sem_proc(ant_name):
        base = ant_name.rsplit("_", 1)[0]
        return name_to_idx.get(base)

    # per-proc streams in tick order
    streams = {}
    all_insts = []
    for blk in nc.m.functions[0].blocks:
        for ins in blk.instructions:
            all_insts.append(ins)
            p = ins.bass_scheduled_proc
            t = ins.bass_scheduled_tick
            if p is not None and t is not None:
                streams.setdefault(p, []).append((t, ins))
    for p in streams:
        streams[p].sort(key=lambda x: x[0])

    def merge(a, b):
        for k, v in b.items():
            if a.get(k, -1) < v:
                a[k] = v

    # K(ins): facts known when ins starts = from its waits.
    # SK(p, i): cumulative facts after first i insts of in-order proc p.
    K_memo, SK_memo = {}, {}

    def K(ins):
        r = K_memo.get(ins.name)
        if r is not None:
            return r
        K_memo[ins.name] = {}  # cycle guard (schedule is acyclic anyway)
        facts = {}
        si = ins.sync_info
        if si is not None:
            for w in si.on_wait:
                if w.wait_mode != "sem-ge-imm" or w.wait_reg is not None:
                    continue
                p = sem_proc(w.ant_name)
                if p is None:
                    continue
                t = w.wait_value // INC[p]
                merge(facts, {p: t})
                if p not in dma_procs:
                    merge(facts, SK(p, t))
                else:
                    # lane sem >= 16t implies the first t DMAs on the lane
                    # started (per-lane FIFO), hence their waits held
                    merge(facts, DK(p, t))
        K_memo[ins.name] = facts
        return facts

    def SK(p, t):
        # facts after in-order proc p's stream reached tick t
        st = streams.get(p, [])
        i = 0
        while i < len(st) and st[i][0] <= t:
            i += 1
        key = (p, i)
        r = SK_memo.get(key)
        if r is not None:
            return r
        SK_memo[key] = {}  # cycle guard
        if i == 0:
            facts = {}
        else:
            tick_i, ins_i = st[i - 1]
            facts = dict(SK(p, tick_i - 1))
            merge(facts, K(ins_i))
            merge(facts, {p: tick_i})
        SK_memo[key] = facts
        return facts

    def DK(p, t):
        # facts guaranteed when in-order proc p DISPATCHES past tick t:
        # union of K(inst_i) for tick_i <= t (no completion facts; earlier
        # instructions' waits were satisfied at their dispatch, even if
        # they have not completed yet)
        st = streams.get(p, [])
        i = 0
        while i < len(st) and st[i][0] <= t:
            i += 1
        key = ("DK", p, i)
        r = SK_memo.get(key)
        if r is not None:
            return r
        SK_memo[key] = {}
        if i == 0:
            facts = {}
        else:
            tick_i, ins_i = st[i - 1]
            facts = dict(DK(p, tick_i - 1))
            merge(facts, K(ins_i))
        SK_memo[key] = facts
        return facts

    def prune_inst(ins, strict_one):
        si = ins.sync_info
        if si is None or len(si.on_wait) <= 1:
            return 0
        waits = list(si.on_wait)
        if any(w.wait_mode != "sem-ge-imm" or w.wait_reg is not None
               for w in waits):
            if strict_one:
                raise RuntimeError(f"{ins.name}: non-imm wait on {ins.name}")
            return 0
        # implicit knowledge from same-proc program order
        base = {}
        p0, t0 = ins.bass_scheduled_proc, ins.bass_scheduled_tick
        if p0 is not None and t0 is not None and p0 not in dma_procs:
            base = dict(DK(p0, t0 - 1))
            # DVE/ACT/PE execute and complete strictly in order with an
            # output-hazard interlock, so same-engine RAW data is visible
            # without an explicit sem wait
            # Pool (gpsimd) qualifies for partition-aligned elementwise
            # chains: each Q7 core runs its instruction stream in order and
            # only touches its own 16 partitions
            if PROC_NAMES[p0] in ("DVE", "Activation", "PE", "Pool"):
                merge(base, SK(p0, t0 - 1))
                merge(base, {p0: t0 - 1})
        # iteratively drop any wait implied by base + the other waits
        kept = list(waits)
        changed = True
        while changed and len(kept) > 1:
            changed = False
            for w in list(kept):
                others = [o for o in kept if o is not w]
                facts = dict(base)
                for o in others:
                    p = sem_proc(o.ant_name)
                    if p is None:
                        continue
                    t = o.wait_value // INC[p]
                    merge(facts, {p: t})
                    if p not in dma_procs:
                        merge(facts, SK(p, t))
                    else:
                        merge(facts, DK(p, t))
                pw = sem_proc(w.ant_name)
                tw = w.wait_value // INC[pw] if pw is not None else None
                if pw is not None and facts.get(pw, -1) >= tw:
                    kept = others
                    changed = True
                    break
        if strict_one and len(kept) > 1:
            raise RuntimeError(
                f"{ins.name} ({type(ins).__name__}): cannot reduce waits to "
                "one: " + str([(w.ant_name, w.wait_value) for w in kept]))
        if len(kept) < len(waits):
            si.on_wait = kept
            ins.sync_info = si
            return 1
        return 0

    n_pruned = 0
    for ins in all_insts:
        if not ins.is_executable():
            continue
        n_pruned += prune_inst(ins, strict_one=not ins.is_sequencer_only())
    return n_pruned


def _build_bass():
    nc = bass.Bass("TRN2", target_bir_lowering=False, debug=False,
                   num_devices=NCORES)

    xp = nc.dram_tensor("xp", [BL, C, HP, WP], F32, kind="ExternalInput").ap()
    w2t = nc.dram_tensor("w2t", [2, CG, P, C], F32, kind="ExternalInput").ap()
    b2 = nc.dram_tensor("b2", [2, C], F32, kind="ExternalInput").ap()
    wmx = nc.dram_tensor("wmx", [C, NTAPS], F32, kind="ExternalInput").ap()
    gam = nc.dram_tensor("gam", [C], F32, kind="ExternalInput").ap()
    bet = nc.dram_tensor("bet", [C], F32, kind="ExternalInput").ap()
    out = nc.dram_tensor("out", [BL, C, H, W], F32, kind="ExternalOutput").ap()

    with tile.TileContext(nc) as tc, ExitStack() as ctx:
        singles = ctx.enter_context(tc.tile_pool(name="singles", bufs=1))
        xtp = ctx.enter_context(tc.tile_pool(name="xtp", bufs=4))
        accp = ctx.enter_context(tc.tile_pool(name="accp", bufs=3))
        tmpp = ctx.enter_context(tc.tile_pool(name="tmpp", bufs=3))
        small = ctx.enter_context(tc.tile_pool(name="small", bufs=4))
        psum = ctx.enter_context(tc.tile_pool(name="psum", bufs=2, space="PSUM"))

        # ---- load params (once) ----
        w2t_sb = singles.tile([P, 2, CG, C], F32)
        nc.sync.dma_start(out=w2t_sb, in_=w2t.rearrange("s g p c -> p s g c"))
        b2_sb = singles.tile([P, 2, CG], F32)
        nc.sync.dma_start(out=b2_sb, in_=b2.rearrange("s (g p) -> p s g", p=P))
        wmx_sb = singles.tile([P, CG, NTAPS], F32)
        nc.sync.dma_start(out=wmx_sb, in_=wmx.rearrange("(g p) t -> p g t", p=P))
        gam_sb = singles.tile([P, CG], F32)
        nc.sync.dma_start(out=gam_sb, in_=gam.rearrange("(g p) -> p g", p=P))
        bet_sb = singles.tile([P, CG], F32)
        nc.sync.dma_start(out=bet_sb, in_=bet.rearrange("(g p) -> p g", p=P))

        # DVE observers of param loads: thread each load's completion into
        # DVE's knowledge so downstream instruction waits collapse to one sem
        # (ISA instructions carry at most one sync wait)
        w2t_obs = singles.tile([P, 3], F32)
        nc.vector.tensor_copy(out=w2t_obs[:, 0:1], in_=w2t_sb[:, 0, 0, 0:1])
        nc.vector.tensor_copy(out=w2t_obs[:, 1:2], in_=gam_sb[:, 0:1])
        nc.vector.tensor_copy(out=w2t_obs[:, 2:3], in_=bet_sb[:, 0:1])

        # bias sum b_avg + b_max  [P, CG]
        bsum = singles.tile([P, CG], F32)
        nc.vector.tensor_add(out=bsum, in0=b2_sb[:, 0, :], in1=b2_sb[:, 1, :])

        # row norms of w_mix: wn[c] = sqrt(sum_t w_mix[c,t]^2)  [P, CG]
        wn = singles.tile([P, CG], F32)
        for g in range(CG):
            sq = small.tile([P, NTAPS], F32, tag="sq49")
            nc.vector.tensor_mul(out=sq, in0=wmx_sb[:, g, :], in1=wmx_sb[:, g, :])
            nc.vector.tensor_reduce(out=wn[:, g:g + 1], in_=sq, axis=AX.X, op=OP.add)
        nc.scalar.activation(out=wn, in_=wn, func=AF.Sqrt)

        # ---- phase 1: pooling over each [P, HP*WP] tile ----
        # X2[p, s, g, b]: s=0 mean, s=1 max
        x2 = singles.tile([P, 2, CG, BL], F32)
        for b in range(BL):
            for g in range(CG):
                xt = xtp.tile([P, HP, WP], F32, tag="xt")
                nc.sync.dma_start(out=xt, in_=xp[b, g * P:(g + 1) * P, :, :])
                ssum = small.tile([P, 1], F32, tag="ssum")
                # in-place identity copy on ACT; the free-dim sum falls out
                # of accum_out, keeping the spatial sum off the DVE
                nc.scalar.activation(out=xt, in_=xt, func=AF.Copy,
                                     accum_out=ssum)
                nc.vector.tensor_scalar_mul(x2[:, 0, g, b:b + 1], ssum, 1.0 / (H * W))
                nc.vector.tensor_reduce(
                    out=x2[:, 1, g, b:b + 1],
                    in_=xt[:, PAD:PAD + H, PAD:PAD + W], axis=AX.XY, op=OP.max)

        # ---- phase 2: stats -> kern ----
        xw = singles.tile([P, CG, BL], F32)
        gx = singles.tile([P, CG, BL], F32)
        for m in range(CG):  # output-channel group
            ps = psum.tile([P, BL], F32, tag="ps_mm")
            k = 0
            for s in range(2):
                for g in range(CG):
                    nc.tensor.matmul(
                        ps, w2t_sb[:, s, g, m * P:(m + 1) * P], x2[:, s, g, :],
                        start=(k == 0), stop=(k == 5))
                    k += 1
            # x_w = gelu(mm + b_avg + b_max)
            nc.scalar.activation(out=xw[:, m, :], in_=ps, func=AF.Gelu,
                                 bias=bsum[:, m:m + 1], scale=1.0)
            # Gx = |x_w| * wn  (wn > 0)
            nc.scalar.activation(out=gx[:, m, :], in_=xw[:, m, :], func=AF.Abs,
                                 scale=wn[:, m:m + 1])

        # mean over channels via PE: ones^T @ gx sums partitions
        ones_col = singles.tile([P, 1], F32)
        nc.vector.memset(ones_col, 1.0)
        ones_row = singles.tile([1, P], F32)
        nc.vector.memset(ones_row, 1.0)
        ps_sum = psum.tile([1, CG * BL], F32, tag="ps_sum")
        nc.tensor.matmul(ps_sum, ones_col, gx.rearrange("p g b -> p (g b)"),
                         start=True, stop=True)
        gsum_sb = small.tile([1, CG, BL], F32, tag="gsum")
        nc.vector.tensor_copy(out=gsum_sb.rearrange("p g b -> p (g b)"),
                              in_=ps_sum)
        rb1 = small.tile([1, BL], F32, tag="rb1")
        nc.vector.tensor_reduce(out=rb1, in_=gsum_sb.rearrange("p g b -> p b g"),
                                axis=AX.X, op=OP.add)
        # r = 1 / (mean + eps)
        nc.vector.tensor_scalar(out=rb1, in0=rb1, scalar1=1.0 / C,
                                scalar2=EPS, op0=OP.mult, op1=OP.add)
        nc.vector.reciprocal(out=rb1, in_=rb1)
        # broadcast r to all partitions: ones_row^T @ rb1 -> [P, BL] in PSUM
        rb = psum.tile([P, BL], F32, tag="ps_rb")
        nc.tensor.matmul(rb, ones_row, rb1, start=True, stop=True)

        # s = gamma * x_w * Gx * r ; kern = w_mix * s + beta
        kern = singles.tile([P, CG, BL, NTAPS], F32)
        for m in range(CG):
            t = small.tile([P, BL], F32, tag="s_tmp")
            nc.vector.tensor_mul(out=t, in0=xw[:, m, :], in1=gx[:, m, :])
            nc.vector.tensor_mul(out=t, in0=t, in1=rb)
            nc.vector.tensor_scalar_mul(t, t, gam_sb[:, m:m + 1])
            for b in range(BL):
                nc.vector.tensor_scalar(
                    out=kern[:, m, b, :], in0=wmx_sb[:, m, :],
                    scalar1=t[:, b:b + 1], scalar2=bet_sb[:, m:m + 1],
                    op0=OP.mult, op1=OP.add)

        # ---- phase 3: depthwise conv, 49 shifted MAC taps ----
        # 3-way engine split: DVE runs fused STT taps; ACT pre-scales taps
        # (per-partition scale) into tmp tiles; GPSIMD accumulates those with
        # plain tensor_tensor adds into its own accumulator.
        ACT_TAPS = list(range(1, 2 * 16, 2))   # 16 taps via ACT+GPS
        for b in range(BL):
            for g in range(CG):
                xt = xtp.tile([P, HP, WP], F32, tag="xt")
                nc.sync.dma_start(out=xt, in_=xp[b, g * P:(g + 1) * P, :, :])
                # per-engine observers of the load: collapse downstream waits
                obs_v = small.tile([P, 1], F32, tag="obs_v")
                nc.vector.tensor_copy(out=obs_v, in_=xt[:, 0, 0:1])
                obs_a = small.tile([P, 1], F32, tag="obs_a")
                nc.scalar.copy(out=obs_a, in_=xt[:, 0, 0:1])
                # ACT observer of this tile's kern column (DVE-written)
                obs_k = small.tile([P, 1], F32, tag="obs_k")
                nc.scalar.copy(out=obs_k, in_=kern[:, g, b, 0:1])
                acc = accp.tile([P, H, W], F32, tag="acc")
                accg = accp.tile([P, H, W], F32, tag="accg")
                first_v, first_g = True, True
                tmp = None
                for t in range(NTAPS):
                    di, dj = t // KW, t % KW
                    xv = xt[:, di:di + H, dj:dj + W]
                    ks = kern[:, g, b, t:t + 1]
                    if t in ACT_TAPS:
                        tmp = tmpp.tile([P, H, W], F32, tag="tmp_act")
                        nc.scalar.mul(out=tmp, in_=xv, mul=ks)
                        if first_g:
                            nc.gpsimd.tensor_copy(out=accg, in_=tmp)
                            first_g = False
                        else:
                            nc.gpsimd.tensor_add(out=accg, in0=accg, in1=tmp)
                    elif first_v:
                        nc.vector.tensor_scalar(out=acc, in0=xv, scalar1=ks,
                                                scalar2=None, op0=OP.mult)
                        first_v = False
                    else:
                        nc.vector.scalar_tensor_tensor(
                            out=acc, in0=xv, scalar=ks, in1=acc,
                            op0=OP.mult, op1=OP.add)
                # combine on gpsimd: accg stays engine-local
                nc.gpsimd.tensor_add(out=acc, in0=acc, in1=accg)
                # ACT tail observer, then a DVE join reading xt + the ACT
                # tail: the slot-recycling load then needs only one wait
                obs_a2 = small.tile([P, 1], F32, tag="obs_a2")
                nc.scalar.copy(out=obs_a2, in_=tmp[:, 0, 0:1])
                nc.vector.tensor_add(out=obs_v, in0=xt[:, 0, 0:1],
                                     in1=obs_a2)
                nc.sync.dma_start(out=out[b, g * P:(g + 1) * P, :, :],
                                  in_=acc)
                # DVE observer of the store completion for slot recycling
                nc.vector.memset(acc[:, 0, 0:1], 0.0)

    import sys
    sys.setrecursionlimit(100000)
    _prune_redundant_dma_waits(nc)
    return nc


_NC_CACHE = {}


def _get_nc():
    if "nc" not in _NC_CACHE:
        _NC_CACHE["nc"] = _build_bass()
    return _NC_CACHE["nc"]


def _prep_inputs(x, w_avg, b_avg, w_max, b_max, w_mix, gamma, beta):
    x = np.asarray(x, dtype=np.float32)
    xp = np.zeros((B, C, HP, WP), dtype=np.float32)
    xp[:, :, PAD:PAD + H, PAD:PAD + W] = x
    w2t = np.ascontiguousarray(
        np.stack([np.asarray(w_avg, np.float32).T.reshape(CG, P, C),
                  np.asarray(w_max, np.float32).T.reshape(CG, P, C)]))
    b2 = np.ascontiguousarray(
        np.stack([np.asarray(b_avg, np.float32), np.asarray(b_max, np.float32)]))
    shared = {
        "w2t": w2t,
        "b2": b2,
        "wmx": np.ascontiguousarray(np.asarray(w_mix, np.float32)),
        "gam": np.ascontiguousarray(np.asarray(gamma, np.float32).reshape(C)),
        "bet": np.ascontiguousarray(np.asarray(beta, np.float32).reshape(C)),
    }
    in_maps = []
    for i in range(NCORES):
        m = dict(shared)
        m["xp"] = np.ascontiguousarray(xp[i * BL:(i + 1) * BL])
        in_maps.append(m)
    return in_maps


def run(inputs, trace=False):
    nc = _get_nc()
    in_maps = _prep_inputs(**inputs)
    res = bass_utils.run_bass_kernel_spmd(
        nc, in_maps, core_ids=list(range(NCORES)), trace=trace)
    outs = [res.results[i]["out"] for i in range(NCORES)]
    full = np.concatenate(outs, axis=0).astype(np.float32)
    return full, res


def kernel(**inputs) -> np.ndarray:
    full, _ = run(inputs, trace=False)
    return full
